# revision 28
# baseline (speedup 1.0000x reference)
"""Trainium2 Bass kernel for nn_Attention_Emb (dense transformer attention
with embedding-selected QKV projections and a relative-position branch).

Sharding: 16 (batch, head) units, 2 per core across 8 NeuronCores.

Math notes (exact reductions, no approximation beyond low-precision matmuls):
- pos_attn[b,h,s,t] = softmax_t((ph[s]-ph[t])@hw + hb) = softmax_t(-ph[t]@hw)
  is independent of s (shift invariance) -> a single row p[t] per (b,h);
  its output contribution is the rank-1 term pvw = p @ VW, computed on HOST
  (pvw = (xu@p)^T @ Rvw, O(S*d) per unit).
- softmax over t of ((k0[t]+s)@(q0[s]+s)) == softmax over t of
  (k0[t]@q0[s] + k0[t]@s) (s-only terms cancel); the t-dependent part
  c[t] = k0[t]@s/sqrt(hd) is computed on HOST (c = xu^T @ (Lk@s)/sqrt(hd))
  and applied as the per-partition bias of the device exp.
- v = v0 + s with sum_t attn = 1 -> the +s contribution is a constant bias
  (host, cb0).
- final renormalization divides by sum((1-g)*A + g*P) == 1 exactly.
- the output projection is fused into the v projection:
  VW[t,m] = sum_d v0[t,d]*OW[d,m]  via rhs = blkdiag((out_w @ ve).T).

Device per unit, t-on-partitions score layout:
  Q = (blkdiag(qe.T)/sqrt(hd)) @ xu, K0 = blkdiag(ke.T) @ xu,
  VW[t,m] (fp8, host-scaled), E[t,s] = exp(K0[:,t]@Q[:,s] + c[t]) in fp8,
  M1[m,s] = sum_t VW[t,m] E[t,s] and Z[s] = sum_t E[t,s] via fp8 DoubleRow
  matmuls (contract 256/pass).
Host combines: out = (1-g)/Z * M1/SCALE_V + g*pvw + (blkdiag(out_w.T).T@s + out_b).
"""

import numpy as np
import ml_dtypes

BF16 = ml_dtypes.bfloat16

B, S, W, DIM, HEADS = 4, 1024, 8, 64, 4
HD = 128
NCORES = 8
SQ = (slice(0, 512), slice(512, 1024))

_prog_cache = {}


def _split_multiwait_insts(nc):
    """walrus codegen rejects instructions carrying >1-2 sem waits; move the
    extras onto preceding same-engine NoOps (equivalent: engine executes its
    instructions in program order)."""
    import concourse.mybir as mybir

    for f in nc.m.functions:
        for bb in f.blocks:
            insts = bb.instructions
            i = 0
            while i < len(insts):
                inst = insts[i]
                si = inst.sync_info
                cap = 2 if type(inst).__name__ == "InstEventSemaphoreOp" else 1
                if si is not None and len(si.on_wait) > cap:
                    waits = list(si.on_wait)
                    extra, keep = waits[:-cap], waits[-cap:]
                    new = []
                    for k, w in enumerate(extra):
                        nop = mybir.InstNoOp(
                            name=f"{inst.name}_splitw{k}", ins=[], outs=[]
                        )
                        nop.engine = inst.engine
                        nop.sync_info = mybir.SyncInfo(on_wait=[w], on_update=[])
                        new.append(nop)
                    inst.sync_info = mybir.SyncInfo(
                        on_wait=keep, on_update=list(si.on_update)
                    )
                    insts[i:i] = new
                    i += len(new)
                i += 1


def _build_program(expscale=1.0):
    key = ("nc", float(expscale))
    if key in _prog_cache:
        return _prog_cache[key]
    import concourse.bass as bass
    import concourse.mybir as mybir
    import concourse.tile as tile

    f32 = mybir.dt.float32
    bf16 = mybir.dt.bfloat16
    fp8 = mybir.dt.float8e4
    AF = mybir.ActivationFunctionType
    DR = mybir.MatmulPerfMode.DoubleRow
    ts = bass.ts

    nc = bass.Bass(trn_type="TRN2")
    xu = nc.dram_tensor("xu", [2, 128, S], fp8, kind="ExternalInput")
    wq = nc.dram_tensor("wq", [128, 256], fp8, kind="ExternalInput")  # Lq|Lk
    wv = nc.dram_tensor("wv", [128, 128], fp8, kind="ExternalInput")
    cbi = nc.dram_tensor("cbi", [128, 16], f32, kind="ExternalInput")
    m1o = nc.dram_tensor("m1o", [2, 128, S], bf16, kind="ExternalOutput")
    zo = nc.dram_tensor("zo", [2, 1, 1024], f32, kind="ExternalOutput")

    def _light_drain_and_barrier(self, tick_clock, wait_clock):
        from concourse.vector_clock import ScopedClock

        drain_inst = self.nc.sync.drain()
        wait_clock.add_sem_waits(
            drain_inst.ins, ScopedClock({None: tick_clock.global_clock})
        )
        self.nc.all_engine_barrier()
        popped = self.nc._tile_sem_poison_stack.pop()
        assert popped is self._sem_poison
        self.nc.clear_and_free_semaphores(list(self.sems.allocated().values()))

    orig_dab = tile.TileContext._drain_and_barrier
    tile.TileContext._drain_and_barrier = _light_drain_and_barrier

    with tile.TileContext(nc) as tc:
        with (
            tc.tile_pool(name="wp", bufs=1) as wp,
            tc.tile_pool(name="xp", bufs=1) as xp,
            tc.tile_pool(name="sp", bufs=1) as sp,
            tc.tile_pool(name="op", bufs=1) as op,
            tc.tile_pool(name="pa", bufs=2, space="PSUM") as pa,
            tc.tile_pool(name="pu", bufs=1, space="PSUM") as pu,
        ):
            # PE warm-up: dummy matmuls with no input deps ramp the PE
            # p-state through the DMA-wait window.
            WM1 = wp.tile([128, 1], bf16, name="WM1")
            nc.vector.memset(WM1, 1.0)
            WM2 = wp.tile([128, 512], bf16, name="WM2")
            nc.vector.memset(WM2, 0.0)
            # ACT warmup: loads the Exp table during the DMA window.
            WME = wp.tile([1, 2], f32, name="WME")
            nc.vector.memset(WME, 0.0)
            nc.scalar.activation(WME, WME, AF.Exp)
            with tc.tile_pool(name="pw", bufs=1, space="PSUM") as pw:
                WPP = pw.tile([1, 512], f32, name="WPP")
                for _ in range(4):
                    nc.tensor.matmul(WPP, WM1, WM2, start=True, stop=True)

            if True:
                pv = tc.alloc_tile_pool(name="pv", bufs=1, space="PSUM")
                pzref = [None]
                # ---- input DMAs spread over the 3 DGE queues
                XU0 = xp.tile([128, S], fp8, name="xus0")
                XU1 = xp.tile([128, S], fp8, name="xus1")
                XUs = [XU0, XU1]
                WQ = wp.tile([128, 256], fp8, name="WQ")
                WV = wp.tile([128, 128], fp8, name="WV")
                CBT = wp.tile([128, 16], f32, name="CBT")
                nc.sync.dma_start(out=WQ, in_=wq[:, :])
                nc.sync.dma_start(out=XU0[:, SQ[0]], in_=xu[0][:, SQ[0]])
                nc.scalar.dma_start(out=XU0[:, 512:768], in_=xu[0][:, 512:768])
                nc.scalar.dma_start(out=XU0[:, 768:1024], in_=xu[0][:, 768:1024])
                nc.scalar.dma_start(out=XU1[:, SQ[0]], in_=xu[1][:, SQ[0]])
                nc.gpsimd.dma_start(out=WV, in_=wv[:, :])
                nc.gpsimd.dma_start(out=CBT, in_=cbi[:, :])
                nc.gpsimd.dma_start(out=XU1[:, SQ[1]], in_=xu[1][:, SQ[1]])
                ONES = wp.tile([128, 2, 16], fp8, name="ONES")
                nc.vector.memset(ONES, 1.0)

                QSs, KSs = [None, None], [None, None]
                ACCs, ZTs = [None, None], [None, None]
                ETs = {}
                VWSs = [
                    sp.tile([128, 8, 128], fp8, name="vws0"),
                    sp.tile([128, 8, 128], fp8, name="vws1"),
                ]

                def emit_p1_half(dst, wcol, q, tag):
                    # unit1 Q/K projection half through the 1-bank pv pool so
                    # the score-slot rotation (pa) stays exp-cadence locked
                    PH = pv.tile([128, 512], f32, name=f"p1{tag}{q}", tag="pv")
                    nc.tensor.matmul(
                        PH, WQ[:, wcol : wcol + 128], XU1[:, SQ[q]],
                        start=True, stop=True,
                    )
                    if tag == "q":
                        nc.vector.tensor_scalar_mul(
                            dst[:, SQ[q]], PH, float(expscale)
                        )
                    else:
                        nc.vector.tensor_copy(dst[:, SQ[q]], PH)

                def emit_vq(j, h):
                    # one quarter-pass of the V*out_w projection: 4 chunks
                    # through a single-bank PSUM tile, cast to fp8.
                    XU = XUs[j]
                    VQ = pv.tile([128, 4, 128], f32, name=f"vq{j}_{h}", tag="pv")
                    for i in range(4):
                        nc.tensor.matmul(
                            VQ[:, i, :], XU[:, ts(4 * h + i, 128)], WV,
                            start=True, stop=True,
                        )
                    nc.vector.tensor_copy(
                        VWSs[j][:, 4 * h : 4 * h + 4, :], VQ
                    )

                def emit_av(j, p):
                    if p == 0:
                        ACCs[j] = pu.tile([128, S], f32, name=f"acc{j}", tag="pu")
                    st, spf = (p == 0), (p == 3)
                    ET = ETs[(j, p)]
                    for q in range(2):
                        nc.tensor.matmul(
                            ACCs[j][:, SQ[q]], VWSs[j][:, 2 * p : 2 * p + 2, :],
                            ET[:, :, SQ[q]], start=st, stop=spf, perf_mode=DR,
                        )

                def emit_z(j, p):
                    pz = pzref[0]
                    if p == 0:
                        ZTs[j] = pz.tile([16, S], f32, name=f"zt{j}", tag="pz")
                    st, spf = (p == 0), (p == 3)
                    ET = ETs[(j, p)]
                    for q in range(2):
                        nc.tensor.matmul(
                            ZTs[j][:, SQ[q]], ONES,
                            ET[:, :, SQ[q]], start=st, stop=spf, perf_mode=DR,
                        )

                def emit_out(j, tail=False):
                    MS = op.tile([128, S], bf16, name=f"ms{j}")
                    nc.vector.tensor_copy(MS[:, SQ[0]], ACCs[j][:, SQ[0]])
                    nc.sync.dma_start(out=m1o[j][:, SQ[0]], in_=MS[:, SQ[0]])
                    if tail:
                        nc.scalar.copy(MS[:, SQ[1]], ACCs[j][:, SQ[1]])
                    else:
                        nc.vector.tensor_copy(MS[:, SQ[1]], ACCs[j][:, SQ[1]])
                    if tail:
                        nc.scalar.dma_start(out=m1o[j][:, SQ[1]], in_=MS[:, SQ[1]])
                    else:
                        nc.sync.dma_start(out=m1o[j][:, SQ[1]], in_=MS[:, SQ[1]])

                def emit_zout(j, tail=False):
                    ZS = op.tile([1, 1024], f32, name=f"zs{j}")
                    nc.vector.tensor_copy(ZS[0:1, 0:512], ZTs[j][0:1, 0:512])
                    if tail:
                        nc.scalar.copy(ZS[0:1, 512:1024], ZTs[j][0:1, 512:1024])
                    else:
                        nc.vector.tensor_copy(ZS[0:1, 512:1024], ZTs[j][0:1, 512:1024])
                    nc.sync.dma_start(out=zo[j], in_=ZS)

                # unit0 Q/K projections: QP0 uses a pa slot (first turn),
                # KP0 a pu slot; both cast on the then-idle ACT engine.
                KP0 = pu.tile([128, S], f32, name="kp0", tag="pu")
                QP0 = pa.tile([128, S], f32, name="qp0", tag="pa")
                QS0 = sp.tile([128, S], bf16, name="qs0")
                KS0 = sp.tile([128, S], bf16, name="ks0")
                nc.tensor.matmul(
                    QP0[:, SQ[0]], WQ[:, 0:128], XU0[:, SQ[0]],
                    start=True, stop=True,
                )
                nc.scalar.activation(
                    QS0[:, SQ[0]], QP0[:, SQ[0]], AF.Identity,
                    scale=float(expscale),
                )
                nc.tensor.matmul(
                    KP0[:, SQ[0]], WQ[:, 128:256], XU0[:, SQ[0]],
                    start=True, stop=True,
                )
                nc.vector.tensor_copy(KS0[:, 0:128], KP0[:, 0:128])
                nc.vector.tensor_copy(KS0[:, 128:256], KP0[:, 128:256])
                nc.vector.tensor_copy(KS0[:, 256:512], KP0[:, 256:512])
                nc.tensor.matmul(
                    QP0[:, SQ[1]], WQ[:, 0:128], XU0[:, SQ[1]],
                    start=True, stop=True,
                )
                nc.scalar.activation(
                    QS0[:, SQ[1]], QP0[:, SQ[1]], AF.Identity,
                    scale=float(expscale),
                )
                nc.tensor.matmul(
                    KP0[:, SQ[1]], WQ[:, 128:256], XU0[:, SQ[1]],
                    start=True, stop=True,
                )
                nc.vector.tensor_copy(KS0[:, SQ[1]], KP0[:, SQ[1]])
                QSs[0], KSs[0] = QS0, KS0
                QS1 = sp.tile([128, S], bf16, name="qs1")
                KS1 = sp.tile([128, S], bf16, name="ks1")
                QSs[1], KSs[1] = QS1, KS1

                # main score/exp stream; unit1 prep and AV/Z trail behind
                SC15 = [None]
                for k in range(16):
                    j, c = divmod(k, 8)
                    p, half = divmod(c, 2)
                    if k == 8:
                        # bridge slot: the pz pool's banks are idle until the
                        # first Z matmul; using them for Sc8 lets unit1's
                        # first scores precompute instead of waiting for the
                        # exp6 slot turn (kills the unit-boundary exp gaps)
                        SC = pzref[0].tile([128, S], f32, name="sc_b", tag="pz")
                    else:
                        SC = pa.tile([128, S], f32, name=f"sc{j}_{c}", tag="pa")
                    if k == 15:
                        SC15[0] = SC
                    for q in range(2):
                        nc.tensor.matmul(
                            SC[:, SQ[q]], KSs[j][:, ts(c, 128)],
                            QSs[j][:, SQ[q]], start=True, stop=True,
                        )
                    if half == 0:
                        ETs[(j, p)] = sp.tile([128, 2, S], fp8, name=f"et{j}_{p}")
                    if k == 15:
                        # split the last exp so the tail AV/copy/DMA chain for
                        # the s<512 half starts one exp-half earlier
                        nc.scalar.activation(
                            ETs[(j, p)][:, half, 0:512], SC[:, SQ[0]], AF.Exp,
                            bias=CBT[:, 8 * j + c : 8 * j + c + 1],
                        )
                    else:
                        nc.scalar.activation(
                            ETs[(j, p)][:, half, :], SC, AF.Exp,
                            bias=CBT[:, 8 * j + c : 8 * j + c + 1],
                        )
                    if k == 0:
                        emit_vq(0, 0)
                    if k == 1:
                        emit_vq(0, 1)
                        emit_p1_half(QS1, 0, 0, "q")
                    if k == 2:
                        emit_p1_half(QS1, 0, 1, "q")
                    if k == 3:
                        emit_p1_half(KS1, 128, 0, "k")
                    if k == 4:
                        emit_p1_half(KS1, 128, 1, "k")
                    if k == 5:
                        emit_vq(1, 0)
                    if k == 6:
                        emit_vq(1, 1)
                    if k == 7:
                        pv.release()
                        pzref[0] = tc.alloc_tile_pool(
                            name="pz", bufs=1, space="PSUM"
                        )
                    if k >= 3 and k % 2 == 1 and k < 15:
                        gp = (k - 3) // 2
                        emit_av(gp // 4, gp % 4)
                        if gp % 4 == 3:
                            emit_out(gp // 4)
                    if k == 9:
                        emit_z(0, 0)
                    if k == 10:
                        emit_z(0, 1)
                    if k == 11:
                        emit_z(0, 2)
                    if k == 12:
                        emit_z(0, 3)
                        emit_zout(0)
                    if k == 13:
                        emit_z(1, 0)
                    if k == 15:
                        emit_av(1, 2)
                        emit_z(1, 1)
                # tail: q0-half path first (AV/Z q0, M1 q0 copy + DMA),
                # then the second exp half and the q1 path.
                emit_z(1, 2)
                ET13 = ETs[(1, 3)]
                MS1 = op.tile([128, S], bf16, name="ms1")
                ZS1 = op.tile([1, 1024], f32, name="zs1")
                nc.tensor.matmul(
                    ACCs[1][:, SQ[0]], VWSs[1][:, 6:8, :],
                    ET13[:, :, SQ[0]], start=False, stop=True, perf_mode=DR,
                )
                nc.tensor.matmul(
                    ZTs[1][:, SQ[0]], ONES,
                    ET13[:, :, SQ[0]], start=False, stop=True, perf_mode=DR,
                )
                nc.vector.tensor_copy(MS1[:, SQ[0]], ACCs[1][:, SQ[0]])
                nc.sync.dma_start(out=m1o[1][:, SQ[0]], in_=MS1[:, SQ[0]])
                nc.vector.tensor_copy(ZS1[0:1, 0:512], ZTs[1][0:1, 0:512])
                nc.scalar.activation(
                    ET13[:, 1, 512:1024], SC15[0][:, SQ[1]], AF.Exp,
                    bias=CBT[:, 15:16],
                )
                nc.tensor.matmul(
                    ZTs[1][:, SQ[1]], ONES,
                    ET13[:, :, SQ[1]], start=False, stop=True, perf_mode=DR,
                )
                nc.tensor.matmul(
                    ACCs[1][:, 512:768], VWSs[1][:, 6:8, :],
                    ET13[:, :, 512:768], start=False, stop=True, perf_mode=DR,
                    skip_group_check=True,
                )
                nc.vector.tensor_copy(MS1[:, 512:768], ACCs[1][:, 512:768])
                nc.sync.dma_start(out=m1o[1][:, 512:768], in_=MS1[:, 512:768])
                nc.tensor.matmul(
                    ACCs[1][:, 768:1024], VWSs[1][:, 6:8, :],
                    ET13[:, :, 768:1024], start=False, stop=True, perf_mode=DR,
                    skip_group_check=True,
                )
                nc.scalar.copy(MS1[:, 768:1024], ACCs[1][:, 768:1024])
                nc.scalar.dma_start(out=m1o[1][:, 768:1024], in_=MS1[:, 768:1024])
                nc.vector.tensor_copy(ZS1[0:1, 512:1024], ZTs[1][0:1, 512:1024])
                nc.sync.dma_start(out=zo[1], in_=ZS1)
                pzref[0].release()

    tile.TileContext._drain_and_barrier = orig_dab
    _split_multiwait_insts(nc)
    _prog_cache[key] = nc
    return nc


def _blkdiag(m):
    z = np.zeros((64, 64), np.float32)
    return np.block([[m, z], [z, m]]).astype(np.float32)


def _prep(inputs):
    f32 = np.float32
    x = np.asarray(inputs["x"], f32)
    pos = np.asarray(inputs["pos"], f32)
    strength = np.asarray(inputs["strength"], f32)
    eid = int(np.asarray(inputs["embed_id1"]))
    qe = np.asarray(inputs["q_emb_w"], f32)[eid].reshape(DIM, DIM)
    ke = np.asarray(inputs["k_emb_w"], f32)[eid].reshape(DIM, DIM)
    ve = np.asarray(inputs["v_emb_w"], f32)[eid].reshape(DIM, DIM)
    pos_w1 = np.asarray(inputs["pos_w1"], f32)
    pos_b1 = np.asarray(inputs["pos_b1"], f32)
    pos_w2 = np.asarray(inputs["pos_w2"], f32)
    pos_b2 = np.asarray(inputs["pos_b2"], f32)
    head_w = np.asarray(inputs["head_w"], f32)
    gate = np.asarray(inputs["gate"], f32)
    out_w = np.asarray(inputs["out_w"], f32)
    out_b = np.asarray(inputs["out_b"], f32)
    str_w = np.asarray(inputs["str_w"], f32)
    str_b = np.asarray(inputs["str_b"], f32)

    s_vec = (strength @ str_w.T + str_b).astype(f32)
    s_tiled = np.tile(s_vec, 2).astype(f32)
    rsq = f32(1.0 / np.sqrt(HD))
    Lq = _blkdiag(np.ascontiguousarray(qe.T)) * rsq
    Lk = _blkdiag(np.ascontiguousarray(ke.T))
    sq = f32(224.0 / max(float(np.abs(Lq).max()), 1e-30))
    sk = f32(224.0 / max(float(np.abs(Lk).max()), 1e-30))
    expscale = f32(1.0) / (sq * sk)
    Rvw = _blkdiag(np.ascontiguousarray((out_w @ ve).T))
    Low = _blkdiag(np.ascontiguousarray(out_w.T))
    u_k = (Lk @ s_tiled) * rsq  # [128]; c_unit = xu^T @ u_k

    # relative-position branch: softmax_t((ph[s]-ph[t])@hw + hb) = softmax_t(-ph[t]@hw)
    t1 = np.maximum(pos @ pos_w1.T + pos_b1, 0.0).astype(f32)
    ph = (t1 @ pos_w2.T + pos_b2).astype(f32)  # [B, S, 8]
    a = np.einsum("btd,hd->bht", ph, head_w).astype(f32)  # [B, H, S]
    na = -a
    na = na - na.max(axis=-1, keepdims=True)
    e = np.exp(na)
    pvec = (e / e.sum(axis=-1, keepdims=True)).astype(f32)  # [B, H, S]

    g = (1.0 / (1.0 + np.exp(-gate))).astype(f32)  # [H]

    # per-unit host tensors + fp8 scale for VW
    xus = np.empty((16, 128, S), f32)
    cbs = np.empty((16, 128, 8), f32)
    pvws = np.empty((16, 128), f32)
    vwmax = 0.0
    for u in range(16):
        b, h = divmod(u, HEADS)
        xuf = x[b, :, :, 2 * h : 2 * h + 2].transpose(2, 0, 1).reshape(128, S)
        xus[u] = xuf
        cbs[u] = (xuf.T @ u_k).reshape(8, 128).T
        vw = xuf.T @ Rvw  # [S, 128]
        pvws[u] = pvec[b, h] @ vw
        vwmax = max(vwmax, float(np.abs(vw).max()))
    scale_v = f32(192.0 / max(vwmax, 1e-30))

    FP8 = ml_dtypes.float8_e4m3
    wq_host = np.clip(
        np.concatenate([Lq * sq, Lk * sk], axis=1), -240, 240
    ).astype(FP8)  # [128, 256]
    wv_host = np.clip(Rvw * scale_v, -240, 240).astype(FP8)

    in_maps = []
    for core in range(NCORES):
        u0, u1 = 2 * core, 2 * core + 1
        cbarr = np.concatenate([cbs[u0], cbs[u1]], axis=1)  # [128, 16]
        in_maps.append(
            dict(
                xu=np.clip(xus[u0 : u0 + 2], -240, 240).astype(FP8),
                wq=wq_host,
                wv=wv_host,
                cbi=np.ascontiguousarray(cbarr),
            )
        )
    meta = dict(
        g=g, s_tiled=s_tiled, Low=Low, out_b=out_b,
        pvws=pvws, scale_v=scale_v, expscale=expscale,
    )
    return in_maps, meta


def _post(results, meta):
    f32 = np.float32
    g = meta["g"]
    outb_tiled = np.tile(meta["out_b"], 2).astype(f32)  # [128]
    cb0 = meta["Low"].T @ meta["s_tiled"] + outb_tiled  # [128]
    inv_sv = f32(1.0) / meta["scale_v"]
    out = np.empty((B, S, W, DIM), f32)
    for core in range(NCORES):
        r = results[core]
        for j in range(2):
            u = 2 * core + j
            b, h = divmod(u, HEADS)
            M1 = np.asarray(r["m1o"][j], f32) * inv_sv  # [128, S]
            Z = np.asarray(r["zo"][j], f32).reshape(S)  # [S]
            cb = g[h] * meta["pvws"][u] + cb0  # [128]
            F = (1.0 - g[h]) * M1 / Z[None, :] + cb[:, None]
            out[b, :, 2 * h : 2 * h + 2, :] = F.reshape(2, DIM, S).transpose(2, 0, 1)
    return out


def kernel(**inputs) -> np.ndarray:
    import time

    from concourse.bass_utils import run_bass_kernel_spmd

    in_maps, meta = _prep(inputs)
    nc = _build_program(meta["expscale"])
    try:
        res = run_bass_kernel_spmd(nc, in_maps, core_ids=list(range(NCORES)))
    except Exception:
        # one retry: a previous process can leave a core wedged transiently
        time.sleep(3.0)
        res = run_bass_kernel_spmd(nc, in_maps, core_ids=list(range(NCORES)))
    return _post(res.results, meta)
